# revision 12
# baseline (speedup 1.0000x reference)
"""Kronecker-factored linear layer on 8 TRN2 NeuronCores.

out = x @ W.T,  W = sum_b kron(a[b], s[b]),  shapes:
  x: (4, 2048, 4096) f32 -> rows M=8192, in I=4096
  a: (8, 8, 8), s: (8, 512, 512)  ->  W: (4096, 4096)

Sharding: out-dim sharded over 8 cores. Out index o = ao*512 + k with
ao in 0..8, so core g owns exactly the block ao == g: y_g = x @ W_g.T,
W_g[k, c*512+p] = sum_b a[b,g,c] * s[b,k,p].

Device dataflow (all fp32; matmuls run as float32r = full PE rate):
  1. Synthesis: for each of 32 (c,pch) tiles, accumulate in PSUM
        psum[p,k] += diag(a[b,g,c]) . sT_b[pch][p,k]   (8 matmuls)
     where sT_b[p,k] = s[b,k,p]. Diagonal stationaries are built on host
     from `a` (layout prep of a tiny input, 4 MB/core).
  2. Main: stream xT tiles (host-transposed x), stationary = xT tile
     [i128, m128], moving = W slice [i128, k512], PSUM accumulates y
     tile [m128, k512] over the 32 i-tiles.
"""

import numpy as np

import concourse.bass as bass
from concourse import bacc
import concourse.mybir as mybir
import concourse.tile as tile
from concourse.bass_utils import run_bass_kernel_spmd

F32 = mybir.dt.float32
F32R = mybir.dt.float32r

NCORES = 8
M = 8192            # 4*2048 rows of x
I_DIM = 4096
O_DIM = 4096
F = 8               # kronecker factor count / a-dims
KO = 512            # out-block size (O_DIM//F) == per-core out slice
PI = 512            # in-block size (I_DIM//F)
ITILES = I_DIM // 128      # 32 = (c, pch) tiles: c in 0..8, pch in 0..4
PCH = PI // 128            # 4
MCHUNK = 512
NCHUNKS = M // MCHUNK      # 16
MTPC = MCHUNK // 128       # 4


def _build_program(reps=1):
    nc = bacc.Bacc(None)
    xT = nc.dram_tensor("xT", [128, ITILES, M], F32R, kind="ExternalInput")
    # sd packs sT (first 32 chunks of 512) and the diag stack dg (16 more
    # chunks; dg[q, bc, r] lives at sd[q, 32 + bc//4, (bc%4)*128 + r])
    sd = nc.dram_tensor("sd", [128, F * PCH + 16, KO], F32R, kind="ExternalInput")
    y = nc.dram_tensor("y", [M, KO], F32, kind="ExternalOutput")

    with tile.TileContext(nc) as tc:
      for _rep in range(reps):
        with tc.tile_pool(name="wg", bufs=1) as wgp:
            wg = wgp.tile([128, ITILES, KO], F32R)

            # ---- Phase 1: synthesize W_g (resident, [p, (c,pch), k]) ----
            with (
                tc.tile_pool(name="synth", bufs=1) as sp,
                tc.tile_pool(name="spsum", bufs=2, space="PSUM") as spp,
            ):
                sds = sp.tile([128, F * PCH + 16, KO], F32R)
                nc.sync.dma_start(sds[:], sd[:])
                sts = sds[:, : F * PCH, :]
                dgs = sds[:, F * PCH :, :].rearrange(
                    "p a (b r) -> p (a b) r", r=128
                )
                for c in range(F):
                    for pch in range(PCH):
                        ps = spp.tile([128, KO], F32)
                        for b in range(F):
                            nc.tensor.matmul(
                                ps[:],
                                lhsT=dgs[:, b * F + c, :],
                                rhs=sts[:, b * PCH + pch, :],
                                start=(b == 0),
                                stop=(b == F - 1),
                            )
                        nc.vector.tensor_copy(wg[:, c * PCH + pch, :], ps[:])

            # ---- Phase 2: y = x @ W_g.T, streamed over m ----
            with (
                tc.tile_pool(name="xs", bufs=2) as xp,
                tc.tile_pool(name="yp", bufs=4, space="PSUM") as yp,
                tc.tile_pool(name="yo", bufs=4) as yo,
            ):
                for mc in range(NCHUNKS):
                    xt = xp.tile([128, ITILES, MCHUNK], F32R)
                    nc.gpsimd.dma_start(
                        xt[:], xT[:, :, mc * MCHUNK : (mc + 1) * MCHUNK]
                    )
                    for mt in range(MTPC):
                        ps = yp.tile([128, KO], F32)
                        for it in range(ITILES):
                            nc.tensor.matmul(
                                ps[:],
                                lhsT=xt[
                                    :, it, mt * 128 : (mt + 1) * 128
                                ],
                                rhs=wg[:, it, :],
                                start=(it == 0),
                                stop=(it == ITILES - 1),
                            )
                        yt = yo.tile([128, KO], F32)
                        nc.vector.tensor_copy(yt[:], ps[:])
                        m0 = mc * MCHUNK + mt * 128
                        nc.gpsimd.dma_start(y[m0 : m0 + 128, :], yt[:])
    return nc


_PROG = None


def _prepare_xT(x):
    x2 = np.ascontiguousarray(x, dtype=np.float32).reshape(M, I_DIM)
    # xT dram layout [p, itile, m]: element (p, it, m) = x[m, it*128 + p]
    xTn = np.ascontiguousarray(x2.T).reshape(ITILES, 128, M).transpose(1, 0, 2)
    return np.ascontiguousarray(xTn)


def _prepare_inputs_sd(a, s):
    # sT layout [p, b*PCH+pch, k] = s[b, k, pch*128+p]
    sTn = (
        np.ascontiguousarray(s, dtype=np.float32)
        .transpose(2, 0, 1)                  # [p512, b, k]
        .reshape(PCH, 128, F, KO)            # [pch, p, b, k]
        .transpose(1, 2, 0, 3)               # [p, b, pch, k]
        .reshape(128, F * PCH, KO)
    )
    sTn = np.ascontiguousarray(sTn)
    # dg[g] layout [q, b*F+c, r] = a[b, g, c] * (q == r), packed behind sT
    # in one [128, 48, 512] tensor (one DMA -> one wait on first matmul).
    eye = np.eye(128, dtype=np.float32)
    sds = []
    for g in range(NCORES):
        coeff = np.ascontiguousarray(a[:, g, :], dtype=np.float32).reshape(F * F)
        dgn = eye[:, None, :] * coeff[None, :, None]       # [128, 64, 128]
        sdn = np.concatenate(
            [sTn, dgn.reshape(128, 16, KO)], axis=1
        ).astype(np.float32)
        sds.append(np.ascontiguousarray(sdn))
    return sds


def kernel(x, a, s, _want_results=False, _trace=False):
    global _PROG
    if _PROG is None:
        _PROG = _build_program()
        if not _PROG.is_finalized():
            _PROG.finalize()
    sds = _prepare_inputs_sd(a, s)
    xTn = _prepare_xT(x)
    in_maps = [{"xT": xTn, "sd": sds[g]} for g in range(NCORES)]
    res = run_bass_kernel_spmd(
        _PROG, in_maps, core_ids=list(range(NCORES)), trace=_trace
    )
    y = np.concatenate([res.results[g]["y"] for g in range(NCORES)], axis=1)
    out = y.reshape(4, 2048, O_DIM)
    if _want_results:
        return out, res
    return out


# revision 13
# speedup vs baseline: 1.2677x; 1.2677x over previous
"""Kronecker-factored linear layer on 8 TRN2 NeuronCores.

out = x @ W.T,  W = sum_b kron(a[b], s[b]),  shapes:
  x: (4, 2048, 4096) f32 -> rows M=8192, in I=4096
  a: (8, 8, 8), s: (8, 512, 512)  ->  W: (4096, 4096)

Sharding: out-dim sharded over 8 cores. Out index o = ao*512 + k with
ao in 0..8, so core g owns exactly the block ao == g: y_g = x @ W_g.T,
W_g[k, c*512+p] = sum_b a[b,g,c] * s[b,k,p].

Device dataflow (all fp32; matmuls run as float32r = full PE rate):
  1. Synthesis: for each of 32 (c,pch) tiles, accumulate in PSUM
        psum[p,k] += diag(a[b,g,c]) . sT_b[pch][p,k]   (8 matmuls)
     where sT_b[p,k] = s[b,k,p]. Diagonal stationaries are built on host
     from `a` (layout prep of a tiny input, 4 MB/core).
  2. Main: stream xT tiles (host-transposed x), stationary = xT tile
     [i128, m128], moving = W slice [i128, k512], PSUM accumulates y
     tile [m128, k512] over the 32 i-tiles.
"""

import numpy as np

import concourse.bass as bass
from concourse import bacc
import concourse.mybir as mybir
import concourse.tile as tile
from concourse.bass_utils import run_bass_kernel_spmd

F32 = mybir.dt.float32
F32R = mybir.dt.float32r

NCORES = 8
M = 8192            # 4*2048 rows of x
I_DIM = 4096
O_DIM = 4096
F = 8               # kronecker factor count / a-dims
KO = 512            # out-block size (O_DIM//F) == per-core out slice
PI = 512            # in-block size (I_DIM//F)
ITILES = I_DIM // 128      # 32 = (c, pch) tiles: c in 0..8, pch in 0..4
PCH = PI // 128            # 4
MCHUNK = 256
NCHUNKS = M // MCHUNK      # 32
MTPC = MCHUNK // 128       # 2


def _build_program(reps=1):
    nc = bacc.Bacc(None)
    # xT pre-tiled per m-chunk: [mc, p, it, m] so each chunk DMA is one
    # fully contiguous block (full HBM rate; no 2KB strided runs).
    xT = nc.dram_tensor(
        "xT", [NCHUNKS, 128, ITILES, MCHUNK], F32R, kind="ExternalInput"
    )
    # sd: diag stack dg first ([128,16,512] <-> [128,64,128]), then sT in
    # pch-major blocks [128, 8b, 512k] each, so synthesis can start after
    # the first 6 MB and stream the rest.
    sd = nc.dram_tensor("sd", [128, 16 + F * PCH, KO], F32R, kind="ExternalInput")
    y = nc.dram_tensor("y", [M, KO], F32, kind="ExternalOutput")

    with tile.TileContext(nc) as tc:
      for _rep in range(reps):
        with (
            tc.tile_pool(name="wg", bufs=1) as wgp,
            tc.tile_pool(name="dg", bufs=1) as dgp,
            tc.tile_pool(name="st", bufs=2) as stpool,
            tc.tile_pool(name="spsum", bufs=2, space="PSUM") as spp,
            tc.tile_pool(name="xs", bufs=2) as xp,
            tc.tile_pool(name="yp", bufs=4, space="PSUM") as yp,
            tc.tile_pool(name="yo", bufs=4) as yo,
        ):
            wg = wgp.tile([128, ITILES, KO], F32R)

            # ---- Phase 1: synthesize W_g (pch-major, streamed sT) ----
            dgt = dgp.tile([128, 16, KO], F32R)
            nc.sync.dma_start(dgt[:], sd[:, 0:16, :])
            dgv = dgt[:].rearrange("p a (b r) -> p (a b) r", r=128)
            synth_order = []
            for pch in range(PCH):
                stp = stpool.tile([128, F, KO], F32R)
                nc.sync.dma_start(
                    stp[:], sd[:, 16 + pch * F : 16 + (pch + 1) * F, :]
                )
                for c in range(F):
                    ps = spp.tile([128, KO], F32)
                    for b in range(F):
                        nc.tensor.matmul(
                            ps[:],
                            lhsT=dgv[:, b * F + c, :],
                            rhs=stp[:, b, :],
                            start=(b == 0),
                            stop=(b == F - 1),
                        )
                    it = c * PCH + pch
                    nc.vector.tensor_copy(wg[:, it, :], ps[:])
                    synth_order.append(it)

            # ---- Phase 2: y = x @ W_g.T; accumulate in synthesis order
            # so the first main groups only trail synthesis by ~8 MMs ----
            for mc in range(NCHUNKS):
                xt = xp.tile([128, ITILES, MCHUNK], F32R)
                nc.gpsimd.dma_start(xt[:], xT[mc])
                for mt in range(MTPC):
                    ps = yp.tile([128, KO], F32)
                    for j, it in enumerate(synth_order):
                        nc.tensor.matmul(
                            ps[:],
                            lhsT=xt[:, it, mt * 128 : (mt + 1) * 128],
                            rhs=wg[:, it, :],
                            start=(j == 0),
                            stop=(j == ITILES - 1),
                        )
                    yt = yo.tile([128, KO], F32)
                    nc.vector.tensor_copy(yt[:], ps[:])
                    m0 = mc * MCHUNK + mt * 128
                    nc.gpsimd.dma_start(y[m0 : m0 + 128, :], yt[:])
    return nc


_PROG = None


def _prepare_xT(x):
    x2 = np.ascontiguousarray(x, dtype=np.float32).reshape(M, I_DIM)
    # [mc, p, it, m] with element = x[mc*MCHUNK + m, it*128 + p]
    xt = x2.reshape(NCHUNKS, MCHUNK, ITILES, 128).transpose(0, 3, 2, 1)
    return np.ascontiguousarray(xt)


def _prepare_inputs_sd(a, s):
    # sT block, pch-major: [p, pch*F + b, k] = s[b, k, pch*128+p]
    sTn = (
        np.ascontiguousarray(s, dtype=np.float32)
        .transpose(2, 0, 1)                  # [p512, b, k]
        .reshape(PCH, 128, F, KO)            # [pch, p, b, k]
        .transpose(1, 0, 2, 3)               # [p, pch, b, k]
        .reshape(128, PCH * F, KO)
    )
    # dg[g] [q, b*F+c, r] = a[b, g, c] * (q == r), stored first as
    # [128, 16, 512] (= [128, 64, 128] flattened in groups of 4).
    eye = np.eye(128, dtype=np.float32)
    sds = []
    for g in range(NCORES):
        coeff = np.ascontiguousarray(a[:, g, :], dtype=np.float32).reshape(F * F)
        dgn = eye[:, None, :] * coeff[None, :, None]       # [128, 64, 128]
        sdn = np.concatenate(
            [dgn.reshape(128, 16, KO), sTn], axis=1
        ).astype(np.float32)
        sds.append(np.ascontiguousarray(sdn))
    return sds


def kernel(x, a, s, _want_results=False, _trace=False):
    global _PROG
    if _PROG is None:
        _PROG = _build_program()
        if not _PROG.is_finalized():
            _PROG.finalize()
    sds = _prepare_inputs_sd(a, s)
    xTn = _prepare_xT(x)
    in_maps = [{"xT": xTn, "sd": sds[g]} for g in range(NCORES)]
    res = run_bass_kernel_spmd(
        _PROG, in_maps, core_ids=list(range(NCORES)), trace=_trace
    )
    y = np.concatenate([res.results[g]["y"] for g in range(NCORES)], axis=1)
    out = y.reshape(4, 2048, O_DIM)
    if _want_results:
        return out, res
    return out
